# revision 64
# baseline (speedup 1.0000x reference)
"""Trainium2 Bass kernel for a 2-layer dense-GAT encoder (DGATEncoderGraph).

Contract: kernel(**inputs) takes the FULL unsharded inputs (as produced by
setup_inputs()) and returns the FULL [1, 256] output.

Strategy (8 NeuronCores, SPMD):
  - Row-shard the [N, N] attention maps: core c owns query rows
    [c*512, (c+1)*512). Each core holds adj^T slice [N, 512] key-major in
    SBUF (partition = key j, free = query i), so softmax is a free-dim
    normalization and the attention matrix is produced directly in the
    [K=j, M=i] layout the TensorEngine needs as lhsT -- no transposes.
  - Head projections h = x @ W are computed replicated on every core
    (cheap); e_ij = el_i + er_j uses host-precomputed vel = W @ a_el,
    ver = W @ a_er so el/er come from tiny matmuls.
  - exp has no row-max shift (softmax is shift invariant; |s| is small for
    this operator). Masking is additive: a per-layer maskneg tensor
    (0 where adj>0 else -1e30) is fused into the e=el+er build via one
    scalar_tensor_tensor per key block, so exp() yields exact zeros at
    masked entries with no extra pass. The softmax denominator z falls
    out of the attention matmul via an appended ones-column in the rhs.
  - Layer boundary: each core computes its h2 = h1_slice @ W2 pieces
    locally (rows of h2 only need rows of h1), then per-head AllGathers
    move the bf16 pieces to all cores (layer-2 head h starts as soon as
    its own gather lands).
  - Device reduces max over its own 512 nodes; host takes max over the 8
    core maxima and applies the final [256]x[256,256]+bias matvec.
"""

import numpy as np
import ml_dtypes

bf = ml_dtypes.bfloat16

N, F, D1, H1 = 4096, 256, 128, 4
D2, H2, F2 = 256, 6, 512
NC = 8
S = N // NC          # 512 query rows per core
JB = N // 128        # 32 key blocks
IB = S // 128        # 4 query sub-blocks
CH = 8               # key blocks per dense chunk
NCH = JB // CH       # 4 chunks
NEG = 0.2

_BUILT = None


def _build():
    import concourse.bass as bass
    import concourse.mybir as mybir
    from concourse import bacc
    import concourse.tile as tile
    from concourse.masks import make_identity

    dt = mybir.dt
    f32, b16 = dt.float32, dt.bfloat16
    AF = mybir.ActivationFunctionType
    OP = mybir.AluOpType
    AX = mybir.AxisListType

    nc = bacc.Bacc(None, target_bir_lowering=False, num_devices=NC, name="dgat")

    # ------------- I/O -------------
    adjt_d = nc.dram_tensor("adjt", [N, S], f32, kind="ExternalInput")
    xt_d = nc.dram_tensor("xt", [F, N], b16, kind="ExternalInput")
    xto_d = nc.dram_tensor("xto", [F, S], b16, kind="ExternalInput")
    w1_d = nc.dram_tensor("w1t", [F, H1, D1], b16, kind="ExternalInput")
    w2_d = nc.dram_tensor("w2t", [F2, H2, D2], b16, kind="ExternalInput")
    vel1_d = nc.dram_tensor("vel1", [F, H1], b16, kind="ExternalInput")
    ver1_d = nc.dram_tensor("ver1", [F, H1], b16, kind="ExternalInput")
    vel2_d = nc.dram_tensor("vel2", [F2, H2], b16, kind="ExternalInput")
    ver2_d = nc.dram_tensor("ver2", [F2, H2], b16, kind="ExternalInput")
    acbc_d = nc.dram_tensor("acbc", [2, H1 + H2], f32, kind="ExternalInput")
    omax_d = nc.dram_tensor("omax", [2, 128], f32, kind="ExternalOutput")
    oloc_d = nc.dram_tensor("olocal", [S, D2], f32, kind="ExternalOutput")

    def bcast_ap(ap, parts=128):
        # replicate a [1, ...] DRAM/SBUF AP across `parts` partitions
        return bass.AP(tensor=ap.tensor, offset=ap.offset,
                       ap=[[0, parts]] + list(ap.ap))

    with tile.TileContext(nc) as tc:
        with (
            tc.tile_pool(name="persist", bufs=1) as P1,
            tc.tile_pool(name="dram", bufs=1, space="DRAM") as DR,
            tc.tile_pool(name="pacc", bufs=4, space="PSUM") as PACC,
            tc.tile_pool(name="psmall", bufs=4, space="PSUM") as PS,
            tc.tile_pool(name="small", bufs=4) as SM,
        ):
            # ---------- persistent loads ----------
            # adjT is 8MB: split across 4 engines' DMA queues so the first
            # head's work isn't gated on a single-queue 8MB transfer.
            adjT = P1.tile([128, JB, S], f32)
            adj_r = adjt_d[:].rearrange("(q jb p) i -> p q jb i", q=4, p=128)
            for q, eng in enumerate((nc.sync, nc.gpsimd, nc.scalar,
                                     nc.gpsimd)):
                eng.dma_start(out=adjT[:, q * 8:(q + 1) * 8, :],
                              in_=adj_r[:, q])
            # maskneg: 0 where adj>0, -1e30 where masked (additive pre-exp mask)
            mask = P1.tile([128, JB, S], b16)
            for q in range(4):
                nc.vector.tensor_scalar(
                    out=mask[:, q * 8:(q + 1) * 8, :],
                    in0=adjT[:, q * 8:(q + 1) * 8, :],
                    scalar1=0.0, scalar2=-1e30, op0=OP.is_le, op1=OP.mult)
            w1s = P1.tile([128, 2, H1, D1], b16)
            nc.sync.dma_start(out=w1s, in_=w1_d[:].rearrange(
                "(kb p) h d -> p kb h d", p=128))
            w2s = P1.tile([128, 4, H2, D2], b16)
            nc.sync.dma_start(out=w2s, in_=w2_d[:].rearrange(
                "(kb p) h d -> p kb h d", p=128))
            vel1s = P1.tile([128, 2, H1], b16)
            nc.sync.dma_start(out=vel1s, in_=vel1_d[:].rearrange(
                "(kb p) h -> p kb h", p=128))
            ver1s = P1.tile([128, 2, H1], b16)
            nc.sync.dma_start(out=ver1s, in_=ver1_d[:].rearrange(
                "(kb p) h -> p kb h", p=128))
            vel2s = P1.tile([128, 4, H2], b16)
            nc.sync.dma_start(out=vel2s, in_=vel2_d[:].rearrange(
                "(kb p) h -> p kb h", p=128))
            ver2s = P1.tile([128, 4, H2], b16)
            nc.sync.dma_start(out=ver2s, in_=ver2_d[:].rearrange(
                "(kb p) h -> p kb h", p=128))
            acbc = P1.tile([128, 2, H1 + H2], f32)
            nc.gpsimd.dma_start(out=acbc, in_=bcast_ap(acbc_d[:]))
            ident = P1.tile([128, 128], f32)
            make_identity(nc, ident)

            h1s = P1.tile([128, IB, F2], f32)      # layer-1 output slice

            # collective bounce buffers (partition-major pieces: node=lb*128+p)
            gins = [DR.tile([128, 4, 258], b16, name=f"gin{h}")
                    for h in range(H2)]
            gouts = [DR.tile([NC, 128, 4, 258], b16, addr_space="Shared",
                             name=f"gout{h}") for h in range(H2)]
            er2g = DR.tile([NC, 128, 4, H2], f32, addr_space="Shared")
            el2d = DR.tile([H2, S], b16)

            def attention(layer, h, haug, elbc, er_scalar_of, D, out_cb):
                """dense attention for one head; haug [128, JB, >=D+1] bf16 with
                ones at col D; er_scalar_of(jb) -> [128,1] AP; out_cb(ib, pacc_t)"""
                hi = h if layer == 1 else H1 + h
                pacc_t = [PACC.tile([128, D + 1], f32, name=f"pa{layer}_{h}_{ib}",
                                    tag="pacc") for ib in range(IB)]
                for cg in range(NCH):
                    # m = ac*adj + bc; LeakyRelu is identity here: adj >= 0
                    # and ac, bc are ones by problem spec, so ac*adj+bc >= 1.
                    mt = SM.tile([128, CH, S], b16, name="mt", tag="mt", bufs=2)
                    nc.scalar.activation(
                        out=mt, in_=adjT[:, cg * CH:(cg + 1) * CH, :],
                        func=AF.Prelu,
                        bias=acbc[:, 1, hi:hi + 1],
                        scale=acbc[:, 0, hi:hi + 1], alpha=NEG)
                    et = SM.tile([128, CH, S], b16, name="et", tag="et", bufs=2)
                    for j4 in range(CH):
                        jb = cg * CH + j4
                        nc.vector.scalar_tensor_tensor(
                            out=et[:, j4, :], in0=mask[:, jb, :],
                            scalar=er_scalar_of(jb), in1=elbc,
                            op0=OP.add, op1=OP.add)
                    nc.vector.tensor_mul(et, et, mt)
                    nc.scalar.activation(out=et, in_=et, func=AF.Exp)
                    for j4 in range(CH):
                        jb = cg * CH + j4
                        for ib in range(IB):
                            nc.tensor.matmul(
                                pacc_t[ib][:, :],
                                lhsT=et[:, j4, ib * 128:(ib + 1) * 128],
                                rhs=haug[:, jb, 0:D + 1],
                                start=(jb == 0), stop=(jb == JB - 1))
                for ib in range(IB):
                    out_cb(ib, pacc_t[ib])

            # =================== LAYER 1 ===================
            MID_cm = tc.tile_pool(name="mid", bufs=1)
            MID = MID_cm.__enter__()
            h1t = MID.tile([128, 4, S], b16, name="h1t", bufs=1)
            with (
                tc.tile_pool(name="l1", bufs=1) as L1,
                tc.tile_pool(name="haug1", bufs=2) as HA1,
            ):
                xts = L1.tile([128, 2, N], b16)
                xt_r = xt_d[:].rearrange("(kb p) n -> p kb n", p=128)
                nc.sync.dma_start(out=xts[:, 0, :], in_=xt_r[:, 0])
                nc.gpsimd.dma_start(out=xts[:, 1, :], in_=xt_r[:, 1])
                xtos = L1.tile([128, 2, S], b16)
                nc.sync.dma_start(out=xtos, in_=xto_d[:].rearrange(
                    "(kb p) n -> p kb n", p=128))

                # batched el/er for all 4 heads
                elall = L1.tile([H1, S], b16)
                pel = PS.tile([H1, S], f32, name="pel", tag="ps")
                for kb in range(2):
                    nc.tensor.matmul(pel, lhsT=vel1s[:, kb, :],
                                     rhs=xtos[:, kb, :],
                                     start=(kb == 0), stop=(kb == 1))
                nc.vector.tensor_copy(elall, pel)
                eld = DR.tile([H1, S], b16)
                nc.sync.dma_start(out=eld, in_=elall)
                # er in column layout [p, jb, h]: node jb*128+p, via PE
                ercol = L1.tile([128, JB, H1], f32)
                for g in range(8):
                    per = PS.tile([128, 4, H1], f32, name="per", tag="ps")
                    for j4 in range(4):
                        nb = g * 4 + j4
                        for kb in range(2):
                            nc.tensor.matmul(
                                per[:, j4, :],
                                lhsT=xts[:, kb, nb * 128:(nb + 1) * 128],
                                rhs=ver1s[:, kb, :],
                                start=(kb == 0), stop=(kb == 1))
                    nc.vector.tensor_copy(ercol[:, g * 4:(g + 1) * 4, :], per)

                for h in range(H1):
                    haug = HA1.tile([128, JB, D1 + 2], b16, name="haug",
                                    tag="haug")
                    nc.vector.memset(haug[:, :, D1:D1 + 1], 1.0)
                    # h_nat = x @ w1[h], written bf16 into haug cols 0:D1
                    for ng in range(8):
                        pn = PS.tile([128, 512], f32, name="pn", tag="ps")
                        for n4 in range(4):
                            nb = ng * 4 + n4
                            for kb in range(2):
                                nc.tensor.matmul(
                                    pn[:, n4 * 128:(n4 + 1) * 128],
                                    lhsT=xts[:, kb, nb * 128:(nb + 1) * 128],
                                    rhs=w1s[:, kb, h, :],
                                    start=(kb == 0), stop=(kb == 1))
                        src = pn[:].rearrange("p (a b) -> p a b", a=4)
                        dst = haug[:, ng * 4:(ng + 1) * 4, 0:D1]
                        nc.vector.tensor_copy(dst, src)
                    elbc = SM.tile([128, S], b16, name="elbc", tag="elbc",
                                   bufs=2)
                    nc.gpsimd.dma_start(out=elbc, in_=bcast_ap(eld[h]))

                    def l1_out(ib, pa, h=h):
                        rz = SM.tile([128, 1], f32, name="rz", tag="rz")
                        nc.vector.reciprocal(rz, pa[:, D1:D1 + 1])
                        tmp = SM.tile([128, D1], f32, name="tmp", tag="tmp")
                        nc.vector.tensor_scalar(
                            out=tmp, in0=pa[:, 0:D1], scalar1=rz, scalar2=None,
                            op0=OP.mult)
                        ex = SM.tile([128, D1], f32, name="ex", tag="ex")
                        nc.scalar.activation(out=ex, in_=tmp, func=AF.Exp)
                        nc.vector.tensor_scalar(
                            out=ex, in0=ex, scalar1=-1.0, scalar2=0.0,
                            op0=OP.add, op1=OP.min)
                        nc.vector.tensor_scalar(
                            out=tmp, in0=tmp, scalar1=0.0, scalar2=None,
                            op0=OP.max)
                        nc.vector.tensor_add(
                            h1s[:, ib, h * D1:(h + 1) * D1], ex, tmp)

                    attention(1, h, haug, elbc,
                              lambda jb, h=h: ercol[:, jb, h:h + 1],
                              D1, l1_out)
                    # transpose this head's [S, 128] output slice into h1t
                    for nb in range(4):
                        ptt = PS.tile([128, 128], f32, name="ptt", tag="ps")
                        nc.tensor.transpose(
                            ptt, h1s[:, nb, h * D1:(h + 1) * D1], ident)
                        nc.vector.tensor_copy(
                            h1t[:, h, nb * 128:(nb + 1) * 128], ptt)

            # ============ LAYER BOUNDARY: pieces + AllGather ============
            with tc.tile_pool(name="bnd", bufs=2) as BND:
                # batched el2/er2 for all 6 heads
                el2all = BND.tile([H2, S], b16, name="el2all", bufs=1)
                pe2 = PS.tile([H2, S], f32, name="pe2", tag="ps")
                for kb in range(4):
                    nc.tensor.matmul(pe2, lhsT=vel2s[:, kb, :],
                                     rhs=h1t[:, kb, :],
                                     start=(kb == 0), stop=(kb == 3))
                nc.vector.tensor_copy(el2all, pe2)
                nc.sync.dma_start(out=el2d, in_=el2all)
                # er2 piece in column layout [p, lb, h] (node lb*128+p)
                pr2 = PS.tile([128, 4, H2], f32, name="pr2", tag="ps")
                for nb in range(4):
                    for kb in range(4):
                        nc.tensor.matmul(
                            pr2[:, nb, :],
                            lhsT=h1t[:, kb, nb * 128:(nb + 1) * 128],
                            rhs=ver2s[:, kb, :],
                            start=(kb == 0), stop=(kb == 3))
                er2tmp = BND.tile([128, 4, H2], f32, name="er2tmp", bufs=1)
                nc.vector.tensor_copy(er2tmp, pr2)
                er2d = DR.tile([128, 4, H2], f32)
                nc.sync.dma_start(out=er2d, in_=er2tmp)
                nc.gpsimd.collective_compute(
                    "AllGather", mybir.AluOpType.bypass,
                    replica_groups=[list(range(NC))],
                    ins=[er2d.opt()], outs=[er2g.opt()])
                for h in range(H2):
                    pc = BND.tile([128, 4, 258], b16, name="pc", tag="pc")
                    nc.vector.memset(pc[:, :, 256:257], 1.0)
                    for nb in range(4):
                        pp = PS.tile([128, D2], f32, name="pp", tag="ps")
                        for kb in range(4):
                            nc.tensor.matmul(
                                pp, lhsT=h1t[:, kb, nb * 128:(nb + 1) * 128],
                                rhs=w2s[:, kb, h, :],
                                start=(kb == 0), stop=(kb == 3))
                        nc.vector.tensor_copy(pc[:, nb, 0:D2], pp)
                    nc.sync.dma_start(out=gins[h], in_=pc)
                    nc.gpsimd.collective_compute(
                        "AllGather", mybir.AluOpType.bypass,
                        replica_groups=[list(range(NC))],
                        ins=[gins[h].opt()], outs=[gouts[h].opt()])
            MID_cm.__exit__(None, None, None)

            # =================== LAYER 2 ===================
            with tc.tile_pool(name="haug2", bufs=2) as HA2:
                acc = HA2.tile([128, IB, D2], f32, name="acc", bufs=1)
                er2all = HA2.tile([128, JB, H2], f32, name="er2all", bufs=1)
                nc.gpsimd.dma_start(
                    out=er2all[:].rearrange("p (c lb) h -> p c lb h", lb=4),
                    in_=er2g[:].rearrange("c p lb h -> p c lb h"))
                for h in range(H2):
                    aug2 = HA2.tile([128, JB, 258], b16, name="aug2",
                                    tag="aug2")
                    nc.sync.dma_start(
                        out=aug2[:].rearrange("p (c lb) col -> p c lb col",
                                              lb=4),
                        in_=gouts[h][:].rearrange(
                            "c p lb col -> p c lb col"))
                    elbc2 = SM.tile([128, S], b16, name="elbc2", tag="elbc",
                                    bufs=2)
                    nc.gpsimd.dma_start(out=elbc2, in_=bcast_ap(el2d[h]))

                    def l2_out(ib, pa, h=h):
                        rz = SM.tile([128, 1], f32, name="rz2", tag="rz")
                        nc.vector.reciprocal(rz, pa[:, D2:D2 + 1])
                        if h == 0:
                            nc.vector.tensor_scalar(
                                out=acc[:, ib, :], in0=pa[:, 0:D2],
                                scalar1=rz, scalar2=None, op0=OP.mult)
                        else:
                            nc.vector.scalar_tensor_tensor(
                                out=acc[:, ib, :], in0=pa[:, 0:D2],
                                scalar=rz, in1=acc[:, ib, :],
                                op0=OP.mult, op1=OP.add)

                    attention(2, h, aug2, elbc2,
                              lambda jb, h=h: er2all[:, jb, h:h + 1],
                              D2, l2_out)

                # ============ epilogue: mean, elu, node-max ============
                oloc = HA2.tile([128, IB, D2], f32, name="oloc", bufs=1)
                omax_p = HA2.tile([128, 2, IB], f32, name="omax_p", bufs=1)
                omax = HA2.tile([128, 2], f32, name="omax", bufs=1)
                for ib in range(IB):
                    ex = SM.tile([128, D2], f32, name="ex2", tag="tmp")
                    nc.scalar.activation(out=ex, in_=acc[:, ib, :],
                                         func=AF.Exp, scale=1.0 / H2)
                    nc.vector.tensor_scalar(out=ex, in0=ex, scalar1=-1.0,
                                            scalar2=0.0, op0=OP.add,
                                            op1=OP.min)
                    t2 = SM.tile([128, D2], f32, name="t2", tag="ex")
                    nc.vector.tensor_scalar(out=t2, in0=acc[:, ib, :],
                                            scalar1=1.0 / H2, scalar2=0.0,
                                            op0=OP.mult, op1=OP.max)
                    nc.vector.tensor_add(oloc[:, ib, :], ex, t2)
                nc.sync.dma_start(
                    out=oloc_d[:].rearrange("(ib p) d -> p ib d", p=128),
                    in_=oloc)
                for ib in range(IB):
                    for dh in range(2):
                        ptt = PS.tile([128, 128], f32, name="ptt2", tag="ps")
                        nc.tensor.transpose(
                            ptt, oloc[:, ib, dh * 128:(dh + 1) * 128], ident)
                        nc.vector.tensor_reduce(
                            out=omax_p[:, dh, ib:ib + 1], in_=ptt,
                            axis=AX.X, op=OP.max)
                for dh in range(2):
                    nc.vector.tensor_reduce(
                        out=omax[:, dh:dh + 1], in_=omax_p[:, dh, :],
                        axis=AX.X, op=OP.max)
                nc.sync.dma_start(out=omax_d[:].rearrange("a p -> p a"),
                                  in_=omax)

    nc.compile()
    return nc


def _get_built():
    global _BUILT
    if _BUILT is None:
        _BUILT = _build()
    return _BUILT


def _marshal(x, adj, w1, a1, w2, a2):
    x0 = np.asarray(x, np.float32)[0]
    adj = np.asarray(adj, np.float32)
    w1 = np.asarray(w1, np.float32)
    a1 = np.asarray(a1, np.float32)
    w2 = np.asarray(w2, np.float32)
    a2 = np.asarray(a2, np.float32)
    xt = np.ascontiguousarray(x0.T).astype(bf)
    w1t = np.ascontiguousarray(np.transpose(w1, (1, 0, 2))).astype(bf)
    w2t = np.ascontiguousarray(np.transpose(w2, (1, 0, 2))).astype(bf)
    vel1 = np.einsum('hfd,hd->fh', w1, a1[:, :D1]).astype(bf)
    ver1 = np.einsum('hfd,hd->fh', w1, a1[:, D1:]).astype(bf)
    vel2 = np.einsum('hfd,hd->fh', w2, a2[:, :D2]).astype(bf)
    ver2 = np.einsum('hfd,hd->fh', w2, a2[:, D2:]).astype(bf)
    return x0, adj, xt, w1t, w2t, vel1, ver1, vel2, ver2


def run(trace=False, **inputs):
    from concourse.bass_utils import run_bass_kernel_spmd
    nc = _get_built()
    x0, adj, xt, w1t, w2t, vel1, ver1, vel2, ver2 = _marshal(
        inputs['x'], inputs['adj'], inputs['w1'], inputs['a1'],
        inputs['w2'], inputs['a2'])
    acbc = np.stack([
        np.concatenate([np.asarray(inputs['ac1'], np.float32),
                        np.asarray(inputs['ac2'], np.float32)]),
        np.concatenate([np.asarray(inputs['bc1'], np.float32),
                        np.asarray(inputs['bc2'], np.float32)]),
    ]).astype(np.float32)
    in_maps = []
    for c in range(NC):
        in_maps.append({
            'adjt': np.ascontiguousarray(adj[c * S:(c + 1) * S, :].T),
            'xt': xt,
            'xto': np.ascontiguousarray(xt[:, c * S:(c + 1) * S]),
            'w1t': w1t, 'w2t': w2t,
            'vel1': vel1, 'ver1': ver1, 'vel2': vel2, 'ver2': ver2,
            'acbc': acbc,
        })
    kw = {}
    if trace:
        kw = dict(trace=True, trace_cores=[0])
    res = run_bass_kernel_spmd(nc, in_maps, core_ids=list(range(NC)), **kw)
    omax = np.max(np.stack([r['omax'] for r in res.results]), axis=0)
    omax = omax.reshape(D2)
    out = (omax @ np.asarray(inputs['Wm'], np.float32)
           + np.asarray(inputs['bm'], np.float32))[None, :]
    return out.astype(np.float32), res


def kernel(**inputs) -> np.ndarray:
    out, _ = run(trace=False, **inputs)
    return out


# revision 65
# speedup vs baseline: 1.0546x; 1.0546x over previous
"""Trainium2 Bass kernel for a 2-layer dense-GAT encoder (DGATEncoderGraph).

Contract: kernel(**inputs) takes the FULL unsharded inputs (as produced by
setup_inputs()) and returns the FULL [1, 256] output.

Strategy (8 NeuronCores, SPMD):
  - Row-shard the [N, N] attention maps: core c owns query rows
    [c*512, (c+1)*512). Each core holds adj^T slice [N, 512] key-major in
    SBUF (partition = key j, free = query i), so softmax is a free-dim
    normalization and the attention matrix is produced directly in the
    [K=j, M=i] layout the TensorEngine needs as lhsT -- no transposes.
  - Head projections h = x @ W are computed replicated on every core
    (cheap); e_ij = el_i + er_j uses host-precomputed vel = W @ a_el,
    ver = W @ a_er so el/er come from tiny matmuls.
  - exp has no row-max shift (softmax is shift invariant; |s| is small for
    this operator). Masking is additive: a per-layer maskneg tensor
    (0 where adj>0 else -1e30) is fused into the e=el+er build via one
    scalar_tensor_tensor per key block, so exp() yields exact zeros at
    masked entries with no extra pass. The softmax denominator z falls
    out of the attention matmul via an appended ones-column in the rhs.
  - Layer boundary: each core computes its h2 = h1_slice @ W2 pieces
    locally (rows of h2 only need rows of h1), then per-head AllGathers
    move the bf16 pieces to all cores (layer-2 head h starts as soon as
    its own gather lands).
  - Device reduces max over its own 512 nodes; host takes max over the 8
    core maxima and applies the final [256]x[256,256]+bias matvec.
"""

import numpy as np
import ml_dtypes

bf = ml_dtypes.bfloat16

N, F, D1, H1 = 4096, 256, 128, 4
D2, H2, F2 = 256, 6, 512
NC = 8
S = N // NC          # 512 query rows per core
JB = N // 128        # 32 key blocks
IB = S // 128        # 4 query sub-blocks
CH = 8               # key blocks per dense chunk
NCH = JB // CH       # 4 chunks
NEG = 0.2

_BUILT = None


def _build():
    import concourse.bass as bass
    import concourse.mybir as mybir
    from concourse import bacc
    import concourse.tile as tile
    from concourse.masks import make_identity

    dt = mybir.dt
    f32, b16 = dt.float32, dt.bfloat16
    AF = mybir.ActivationFunctionType
    OP = mybir.AluOpType
    AX = mybir.AxisListType

    nc = bacc.Bacc(None, target_bir_lowering=False, num_devices=NC, name="dgat")

    # ------------- I/O -------------
    adjt_d = nc.dram_tensor("adjt", [N, S], f32, kind="ExternalInput")
    xt_d = nc.dram_tensor("xt", [F, N], b16, kind="ExternalInput")
    xto_d = nc.dram_tensor("xto", [F, S], b16, kind="ExternalInput")
    w1_d = nc.dram_tensor("w1t", [F, H1, D1], b16, kind="ExternalInput")
    w2_d = nc.dram_tensor("w2t", [F2, H2, D2], b16, kind="ExternalInput")
    vel1_d = nc.dram_tensor("vel1", [F, H1], b16, kind="ExternalInput")
    ver1_d = nc.dram_tensor("ver1", [F, H1], b16, kind="ExternalInput")
    vel2_d = nc.dram_tensor("vel2", [F2, H2], b16, kind="ExternalInput")
    ver2_d = nc.dram_tensor("ver2", [F2, H2], b16, kind="ExternalInput")
    acbc_d = nc.dram_tensor("acbc", [2, H1 + H2], f32, kind="ExternalInput")
    omax_d = nc.dram_tensor("omax", [2, 128], f32, kind="ExternalOutput")
    oloc_d = nc.dram_tensor("olocal", [S, D2], f32, kind="ExternalOutput")

    def bcast_ap(ap, parts=128):
        # replicate a [1, ...] DRAM/SBUF AP across `parts` partitions
        return bass.AP(tensor=ap.tensor, offset=ap.offset,
                       ap=[[0, parts]] + list(ap.ap))

    with tile.TileContext(nc) as tc:
        with (
            tc.tile_pool(name="persist", bufs=1) as P1,
            tc.tile_pool(name="dram", bufs=1, space="DRAM") as DR,
            tc.tile_pool(name="pacc", bufs=4, space="PSUM") as PACC,
            tc.tile_pool(name="psmall", bufs=4, space="PSUM") as PS,
            tc.tile_pool(name="small", bufs=4) as SM,
        ):
            # ---------- persistent loads ----------
            # adjT is 8MB: split across 4 engines' DMA queues so the first
            # head's work isn't gated on a single-queue 8MB transfer.
            adjT = P1.tile([128, JB, S], f32)
            adj_r = adjt_d[:].rearrange("(q jb p) i -> p q jb i", q=4, p=128)
            for q, eng in enumerate((nc.sync, nc.gpsimd, nc.scalar,
                                     nc.gpsimd)):
                eng.dma_start(out=adjT[:, q * 8:(q + 1) * 8, :],
                              in_=adj_r[:, q])
            # maskneg: 0 where adj>0, -1e30 where masked (additive pre-exp mask)
            mask = P1.tile([128, JB, S], b16)
            for q in range(4):
                nc.vector.tensor_scalar(
                    out=mask[:, q * 8:(q + 1) * 8, :],
                    in0=adjT[:, q * 8:(q + 1) * 8, :],
                    scalar1=0.0, scalar2=-1e30, op0=OP.is_le, op1=OP.mult)
            w1s = P1.tile([128, 2, H1, D1], b16)
            nc.sync.dma_start(out=w1s, in_=w1_d[:].rearrange(
                "(kb p) h d -> p kb h d", p=128))
            w2s = P1.tile([128, 4, H2, D2], b16)
            nc.sync.dma_start(out=w2s, in_=w2_d[:].rearrange(
                "(kb p) h d -> p kb h d", p=128))
            vel1s = P1.tile([128, 2, H1], b16)
            nc.sync.dma_start(out=vel1s, in_=vel1_d[:].rearrange(
                "(kb p) h -> p kb h", p=128))
            ver1s = P1.tile([128, 2, H1], b16)
            nc.sync.dma_start(out=ver1s, in_=ver1_d[:].rearrange(
                "(kb p) h -> p kb h", p=128))
            vel2s = P1.tile([128, 4, H2], b16)
            nc.sync.dma_start(out=vel2s, in_=vel2_d[:].rearrange(
                "(kb p) h -> p kb h", p=128))
            ver2s = P1.tile([128, 4, H2], b16)
            nc.sync.dma_start(out=ver2s, in_=ver2_d[:].rearrange(
                "(kb p) h -> p kb h", p=128))
            acbc = P1.tile([128, 2, H1 + H2], f32)
            nc.gpsimd.dma_start(out=acbc, in_=bcast_ap(acbc_d[:]))
            ident = P1.tile([128, 128], f32)
            make_identity(nc, ident)

            h1s = P1.tile([128, IB, F2], f32)      # layer-1 output slice

            # collective bounce buffers (partition-major pieces: node=lb*128+p)
            gins = [DR.tile([128, 4, 258], b16, name=f"gin{h}")
                    for h in range(H2)]
            gouts = [DR.tile([NC, 128, 4, 258], b16, addr_space="Shared",
                             name=f"gout{h}") for h in range(H2)]
            er2g = DR.tile([NC, 128, 4, H2], f32, addr_space="Shared")
            el2d = DR.tile([H2, S], b16)

            def attention(layer, h, haug, elbc, er_scalar_of, D, out_cb):
                """dense attention for one head; haug [128, JB, >=D+1] bf16 with
                ones at col D; er_scalar_of(jb) -> [128,1] AP; out_cb(ib, pacc_t)"""
                hi = h if layer == 1 else H1 + h
                pacc_t = [PACC.tile([128, D + 1], f32, name=f"pa{layer}_{h}_{ib}",
                                    tag="pacc") for ib in range(IB)]
                for cg in range(NCH):
                    # m = ac*adj + bc; LeakyRelu is identity here: adj >= 0
                    # and ac, bc are ones by problem spec, so ac*adj+bc >= 1.
                    mt = SM.tile([128, CH, S], b16, name="mt", tag="mt", bufs=2)
                    nc.scalar.activation(
                        out=mt, in_=adjT[:, cg * CH:(cg + 1) * CH, :],
                        func=AF.Prelu,
                        bias=acbc[:, 1, hi:hi + 1],
                        scale=acbc[:, 0, hi:hi + 1], alpha=NEG)
                    et = SM.tile([128, CH, S], b16, name="et", tag="et", bufs=2)
                    for j4 in range(CH):
                        jb = cg * CH + j4
                        nc.vector.scalar_tensor_tensor(
                            out=et[:, j4, :], in0=mask[:, jb, :],
                            scalar=er_scalar_of(jb), in1=elbc,
                            op0=OP.add, op1=OP.add)
                    nc.vector.tensor_mul(et, et, mt)
                    nc.scalar.activation(out=et, in_=et, func=AF.Exp)
                    # ib-outer: 8 consecutive MMs per PSUM bank -- avoids the
                    # per-MM bank cycling that keeps the PE HAM throttled.
                    for ib in range(IB):
                        for j4 in range(CH):
                            jb = cg * CH + j4
                            nc.tensor.matmul(
                                pacc_t[ib][:, :],
                                lhsT=et[:, j4, ib * 128:(ib + 1) * 128],
                                rhs=haug[:, jb, 0:D + 1],
                                start=(jb == 0), stop=(jb == JB - 1))
                for ib in range(IB):
                    out_cb(ib, pacc_t[ib])

            # =================== LAYER 1 ===================
            MID_cm = tc.tile_pool(name="mid", bufs=1)
            MID = MID_cm.__enter__()
            h1t = MID.tile([128, 4, S], b16, name="h1t", bufs=1)
            with (
                tc.tile_pool(name="l1", bufs=1) as L1,
                tc.tile_pool(name="haug1", bufs=2) as HA1,
            ):
                xts = L1.tile([128, 2, N], b16)
                xt_r = xt_d[:].rearrange("(kb p) n -> p kb n", p=128)
                nc.sync.dma_start(out=xts[:, 0, :], in_=xt_r[:, 0])
                nc.gpsimd.dma_start(out=xts[:, 1, :], in_=xt_r[:, 1])
                xtos = L1.tile([128, 2, S], b16)
                nc.sync.dma_start(out=xtos, in_=xto_d[:].rearrange(
                    "(kb p) n -> p kb n", p=128))

                # batched el/er for all 4 heads
                elall = L1.tile([H1, S], b16)
                pel = PS.tile([H1, S], f32, name="pel", tag="ps")
                for kb in range(2):
                    nc.tensor.matmul(pel, lhsT=vel1s[:, kb, :],
                                     rhs=xtos[:, kb, :],
                                     start=(kb == 0), stop=(kb == 1))
                nc.vector.tensor_copy(elall, pel)
                eld = DR.tile([H1, S], b16)
                nc.sync.dma_start(out=eld, in_=elall)
                # er in column layout [p, jb, h]: node jb*128+p, via PE
                ercol = L1.tile([128, JB, H1], f32)
                for g in range(8):
                    per = PS.tile([128, 4, H1], f32, name="per", tag="ps")
                    for j4 in range(4):
                        nb = g * 4 + j4
                        for kb in range(2):
                            nc.tensor.matmul(
                                per[:, j4, :],
                                lhsT=xts[:, kb, nb * 128:(nb + 1) * 128],
                                rhs=ver1s[:, kb, :],
                                start=(kb == 0), stop=(kb == 1))
                    nc.vector.tensor_copy(ercol[:, g * 4:(g + 1) * 4, :], per)

                for h in range(H1):
                    haug = HA1.tile([128, JB, D1 + 2], b16, name="haug",
                                    tag="haug")
                    nc.vector.memset(haug[:, :, D1:D1 + 1], 1.0)
                    # h_nat = x @ w1[h], written bf16 into haug cols 0:D1
                    for ng in range(8):
                        pn = PS.tile([128, 512], f32, name="pn", tag="ps")
                        for n4 in range(4):
                            nb = ng * 4 + n4
                            for kb in range(2):
                                nc.tensor.matmul(
                                    pn[:, n4 * 128:(n4 + 1) * 128],
                                    lhsT=xts[:, kb, nb * 128:(nb + 1) * 128],
                                    rhs=w1s[:, kb, h, :],
                                    start=(kb == 0), stop=(kb == 1))
                        src = pn[:].rearrange("p (a b) -> p a b", a=4)
                        dst = haug[:, ng * 4:(ng + 1) * 4, 0:D1]
                        nc.vector.tensor_copy(dst, src)
                    elbc = SM.tile([128, S], b16, name="elbc", tag="elbc",
                                   bufs=2)
                    nc.gpsimd.dma_start(out=elbc, in_=bcast_ap(eld[h]))

                    def l1_out(ib, pa, h=h):
                        rz = SM.tile([128, 1], f32, name="rz", tag="rz")
                        nc.vector.reciprocal(rz, pa[:, D1:D1 + 1])
                        tmp = SM.tile([128, D1], f32, name="tmp", tag="tmp")
                        nc.vector.tensor_scalar(
                            out=tmp, in0=pa[:, 0:D1], scalar1=rz, scalar2=None,
                            op0=OP.mult)
                        ex = SM.tile([128, D1], f32, name="ex", tag="ex")
                        nc.scalar.activation(out=ex, in_=tmp, func=AF.Exp)
                        nc.vector.tensor_scalar(
                            out=ex, in0=ex, scalar1=-1.0, scalar2=0.0,
                            op0=OP.add, op1=OP.min)
                        nc.vector.tensor_scalar(
                            out=tmp, in0=tmp, scalar1=0.0, scalar2=None,
                            op0=OP.max)
                        nc.vector.tensor_add(
                            h1s[:, ib, h * D1:(h + 1) * D1], ex, tmp)

                    attention(1, h, haug, elbc,
                              lambda jb, h=h: ercol[:, jb, h:h + 1],
                              D1, l1_out)
                    # transpose this head's [S, 128] output slice into h1t
                    for nb in range(4):
                        ptt = PS.tile([128, 128], f32, name="ptt", tag="ps")
                        nc.tensor.transpose(
                            ptt, h1s[:, nb, h * D1:(h + 1) * D1], ident)
                        nc.vector.tensor_copy(
                            h1t[:, h, nb * 128:(nb + 1) * 128], ptt)

            # ============ LAYER BOUNDARY: pieces + AllGather ============
            with tc.tile_pool(name="bnd", bufs=2) as BND:
                # batched el2/er2 for all 6 heads
                el2all = BND.tile([H2, S], b16, name="el2all", bufs=1)
                pe2 = PS.tile([H2, S], f32, name="pe2", tag="ps")
                for kb in range(4):
                    nc.tensor.matmul(pe2, lhsT=vel2s[:, kb, :],
                                     rhs=h1t[:, kb, :],
                                     start=(kb == 0), stop=(kb == 3))
                nc.vector.tensor_copy(el2all, pe2)
                nc.sync.dma_start(out=el2d, in_=el2all)
                # er2 piece in column layout [p, lb, h] (node lb*128+p)
                pr2 = PS.tile([128, 4, H2], f32, name="pr2", tag="ps")
                for nb in range(4):
                    for kb in range(4):
                        nc.tensor.matmul(
                            pr2[:, nb, :],
                            lhsT=h1t[:, kb, nb * 128:(nb + 1) * 128],
                            rhs=ver2s[:, kb, :],
                            start=(kb == 0), stop=(kb == 3))
                er2tmp = BND.tile([128, 4, H2], f32, name="er2tmp", bufs=1)
                nc.vector.tensor_copy(er2tmp, pr2)
                er2d = DR.tile([128, 4, H2], f32)
                nc.sync.dma_start(out=er2d, in_=er2tmp)
                nc.gpsimd.collective_compute(
                    "AllGather", mybir.AluOpType.bypass,
                    replica_groups=[list(range(NC))],
                    ins=[er2d.opt()], outs=[er2g.opt()])
                for h in range(H2):
                    pc = BND.tile([128, 4, 258], b16, name="pc", tag="pc")
                    nc.vector.memset(pc[:, :, 256:257], 1.0)
                    for nb in range(4):
                        pp = PS.tile([128, D2], f32, name="pp", tag="ps")
                        for kb in range(4):
                            nc.tensor.matmul(
                                pp, lhsT=h1t[:, kb, nb * 128:(nb + 1) * 128],
                                rhs=w2s[:, kb, h, :],
                                start=(kb == 0), stop=(kb == 3))
                        nc.vector.tensor_copy(pc[:, nb, 0:D2], pp)
                    nc.sync.dma_start(out=gins[h], in_=pc)
                    nc.gpsimd.collective_compute(
                        "AllGather", mybir.AluOpType.bypass,
                        replica_groups=[list(range(NC))],
                        ins=[gins[h].opt()], outs=[gouts[h].opt()])
            MID_cm.__exit__(None, None, None)

            # =================== LAYER 2 ===================
            with tc.tile_pool(name="haug2", bufs=2) as HA2:
                acc = HA2.tile([128, IB, D2], f32, name="acc", bufs=1)
                er2all = HA2.tile([128, JB, H2], f32, name="er2all", bufs=1)
                nc.gpsimd.dma_start(
                    out=er2all[:].rearrange("p (c lb) h -> p c lb h", lb=4),
                    in_=er2g[:].rearrange("c p lb h -> p c lb h"))
                for h in range(H2):
                    aug2 = HA2.tile([128, JB, 258], b16, name="aug2",
                                    tag="aug2")
                    nc.sync.dma_start(
                        out=aug2[:].rearrange("p (c lb) col -> p c lb col",
                                              lb=4),
                        in_=gouts[h][:].rearrange(
                            "c p lb col -> p c lb col"))
                    elbc2 = SM.tile([128, S], b16, name="elbc2", tag="elbc",
                                    bufs=2)
                    nc.gpsimd.dma_start(out=elbc2, in_=bcast_ap(el2d[h]))

                    def l2_out(ib, pa, h=h):
                        rz = SM.tile([128, 1], f32, name="rz2", tag="rz")
                        nc.vector.reciprocal(rz, pa[:, D2:D2 + 1])
                        if h == 0:
                            nc.vector.tensor_scalar(
                                out=acc[:, ib, :], in0=pa[:, 0:D2],
                                scalar1=rz, scalar2=None, op0=OP.mult)
                        else:
                            nc.vector.scalar_tensor_tensor(
                                out=acc[:, ib, :], in0=pa[:, 0:D2],
                                scalar=rz, in1=acc[:, ib, :],
                                op0=OP.mult, op1=OP.add)

                    attention(2, h, aug2, elbc2,
                              lambda jb, h=h: er2all[:, jb, h:h + 1],
                              D2, l2_out)

                # ============ epilogue: mean, elu, node-max ============
                oloc = HA2.tile([128, IB, D2], f32, name="oloc", bufs=1)
                omax_p = HA2.tile([128, 2, IB], f32, name="omax_p", bufs=1)
                omax = HA2.tile([128, 2], f32, name="omax", bufs=1)
                for ib in range(IB):
                    ex = SM.tile([128, D2], f32, name="ex2", tag="tmp")
                    nc.scalar.activation(out=ex, in_=acc[:, ib, :],
                                         func=AF.Exp, scale=1.0 / H2)
                    nc.vector.tensor_scalar(out=ex, in0=ex, scalar1=-1.0,
                                            scalar2=0.0, op0=OP.add,
                                            op1=OP.min)
                    t2 = SM.tile([128, D2], f32, name="t2", tag="ex")
                    nc.vector.tensor_scalar(out=t2, in0=acc[:, ib, :],
                                            scalar1=1.0 / H2, scalar2=0.0,
                                            op0=OP.mult, op1=OP.max)
                    nc.vector.tensor_add(oloc[:, ib, :], ex, t2)
                nc.sync.dma_start(
                    out=oloc_d[:].rearrange("(ib p) d -> p ib d", p=128),
                    in_=oloc)
                for ib in range(IB):
                    for dh in range(2):
                        ptt = PS.tile([128, 128], f32, name="ptt2", tag="ps")
                        nc.tensor.transpose(
                            ptt, oloc[:, ib, dh * 128:(dh + 1) * 128], ident)
                        nc.vector.tensor_reduce(
                            out=omax_p[:, dh, ib:ib + 1], in_=ptt,
                            axis=AX.X, op=OP.max)
                for dh in range(2):
                    nc.vector.tensor_reduce(
                        out=omax[:, dh:dh + 1], in_=omax_p[:, dh, :],
                        axis=AX.X, op=OP.max)
                nc.sync.dma_start(out=omax_d[:].rearrange("a p -> p a"),
                                  in_=omax)

    nc.compile()
    return nc


def _get_built():
    global _BUILT
    if _BUILT is None:
        _BUILT = _build()
    return _BUILT


def _marshal(x, adj, w1, a1, w2, a2):
    x0 = np.asarray(x, np.float32)[0]
    adj = np.asarray(adj, np.float32)
    w1 = np.asarray(w1, np.float32)
    a1 = np.asarray(a1, np.float32)
    w2 = np.asarray(w2, np.float32)
    a2 = np.asarray(a2, np.float32)
    xt = np.ascontiguousarray(x0.T).astype(bf)
    w1t = np.ascontiguousarray(np.transpose(w1, (1, 0, 2))).astype(bf)
    w2t = np.ascontiguousarray(np.transpose(w2, (1, 0, 2))).astype(bf)
    vel1 = np.einsum('hfd,hd->fh', w1, a1[:, :D1]).astype(bf)
    ver1 = np.einsum('hfd,hd->fh', w1, a1[:, D1:]).astype(bf)
    vel2 = np.einsum('hfd,hd->fh', w2, a2[:, :D2]).astype(bf)
    ver2 = np.einsum('hfd,hd->fh', w2, a2[:, D2:]).astype(bf)
    return x0, adj, xt, w1t, w2t, vel1, ver1, vel2, ver2


def run(trace=False, **inputs):
    from concourse.bass_utils import run_bass_kernel_spmd
    nc = _get_built()
    x0, adj, xt, w1t, w2t, vel1, ver1, vel2, ver2 = _marshal(
        inputs['x'], inputs['adj'], inputs['w1'], inputs['a1'],
        inputs['w2'], inputs['a2'])
    acbc = np.stack([
        np.concatenate([np.asarray(inputs['ac1'], np.float32),
                        np.asarray(inputs['ac2'], np.float32)]),
        np.concatenate([np.asarray(inputs['bc1'], np.float32),
                        np.asarray(inputs['bc2'], np.float32)]),
    ]).astype(np.float32)
    in_maps = []
    for c in range(NC):
        in_maps.append({
            'adjt': np.ascontiguousarray(adj[c * S:(c + 1) * S, :].T),
            'xt': xt,
            'xto': np.ascontiguousarray(xt[:, c * S:(c + 1) * S]),
            'w1t': w1t, 'w2t': w2t,
            'vel1': vel1, 'ver1': ver1, 'vel2': vel2, 'ver2': ver2,
            'acbc': acbc,
        })
    kw = {}
    if trace:
        kw = dict(trace=True, trace_cores=[0])
    res = run_bass_kernel_spmd(nc, in_maps, core_ids=list(range(NC)), **kw)
    omax = np.max(np.stack([r['omax'] for r in res.results]), axis=0)
    omax = omax.reshape(D2)
    out = (omax @ np.asarray(inputs['Wm'], np.float32)
           + np.asarray(inputs['bm'], np.float32))[None, :]
    return out.astype(np.float32), res


def kernel(**inputs) -> np.ndarray:
    out, _ = run(trace=False, **inputs)
    return out


# revision 66
# speedup vs baseline: 1.0886x; 1.0322x over previous
"""Trainium2 Bass kernel for a 2-layer dense-GAT encoder (DGATEncoderGraph).

Contract: kernel(**inputs) takes the FULL unsharded inputs (as produced by
setup_inputs()) and returns the FULL [1, 256] output.

Strategy (8 NeuronCores, SPMD):
  - Row-shard the [N, N] attention maps: core c owns query rows
    [c*512, (c+1)*512). Each core holds adj^T slice [N, 512] key-major in
    SBUF (partition = key j, free = query i), so softmax is a free-dim
    normalization and the attention matrix is produced directly in the
    [K=j, M=i] layout the TensorEngine needs as lhsT -- no transposes.
  - Head projections h = x @ W are computed replicated on every core
    (cheap); e_ij = el_i + er_j uses host-precomputed vel = W @ a_el,
    ver = W @ a_er so el/er come from tiny matmuls.
  - exp has no row-max shift (softmax is shift invariant; |s| is small for
    this operator). Masking is additive: a per-layer maskneg tensor
    (0 where adj>0 else -1e30) is fused into the e=el+er build via one
    scalar_tensor_tensor per key block, so exp() yields exact zeros at
    masked entries with no extra pass. The softmax denominator z falls
    out of the attention matmul via an appended ones-column in the rhs.
  - Layer boundary: each core computes its h2 = h1_slice @ W2 pieces
    locally (rows of h2 only need rows of h1), then per-head AllGathers
    move the bf16 pieces to all cores (layer-2 head h starts as soon as
    its own gather lands).
  - Device reduces max over its own 512 nodes; host takes max over the 8
    core maxima and applies the final [256]x[256,256]+bias matvec.
"""

import numpy as np
import ml_dtypes

bf = ml_dtypes.bfloat16

N, F, D1, H1 = 4096, 256, 128, 4
D2, H2, F2 = 256, 6, 512
NC = 8
S = N // NC          # 512 query rows per core
JB = N // 128        # 32 key blocks
IB = S // 128        # 4 query sub-blocks
CH = 8               # key blocks per dense chunk
NCH = JB // CH       # 4 chunks
NEG = 0.2

_BUILT = None


def _build():
    import concourse.bass as bass
    import concourse.mybir as mybir
    from concourse import bacc
    import concourse.tile as tile
    from concourse.masks import make_identity

    dt = mybir.dt
    f32, b16 = dt.float32, dt.bfloat16
    AF = mybir.ActivationFunctionType
    OP = mybir.AluOpType
    AX = mybir.AxisListType

    nc = bacc.Bacc(None, target_bir_lowering=False, num_devices=NC, name="dgat")

    # ------------- I/O -------------
    adjt_d = nc.dram_tensor("adjt", [N, S], f32, kind="ExternalInput")
    xt_d = nc.dram_tensor("xt", [F, N], b16, kind="ExternalInput")
    xto_d = nc.dram_tensor("xto", [F, S], b16, kind="ExternalInput")
    w1_d = nc.dram_tensor("w1t", [F, H1, D1], b16, kind="ExternalInput")
    w2_d = nc.dram_tensor("w2t", [F2, H2, D2], b16, kind="ExternalInput")
    vel1_d = nc.dram_tensor("vel1", [F, H1], b16, kind="ExternalInput")
    ver1_d = nc.dram_tensor("ver1", [F, H1], b16, kind="ExternalInput")
    vel2_d = nc.dram_tensor("vel2", [F2, H2], b16, kind="ExternalInput")
    ver2_d = nc.dram_tensor("ver2", [F2, H2], b16, kind="ExternalInput")
    acbc_d = nc.dram_tensor("acbc", [2, H1 + H2], f32, kind="ExternalInput")
    omax_d = nc.dram_tensor("omax", [2, 128], f32, kind="ExternalOutput")
    oloc_d = nc.dram_tensor("olocal", [S, D2], f32, kind="ExternalOutput")

    def bcast_ap(ap, parts=128):
        # replicate a [1, ...] DRAM/SBUF AP across `parts` partitions
        return bass.AP(tensor=ap.tensor, offset=ap.offset,
                       ap=[[0, parts]] + list(ap.ap))

    with tile.TileContext(nc) as tc:
        with (
            tc.tile_pool(name="persist", bufs=1) as P1,
            tc.tile_pool(name="dram", bufs=1, space="DRAM") as DR,
            tc.tile_pool(name="pacc", bufs=4, space="PSUM") as PACC,
            tc.tile_pool(name="psmall", bufs=4, space="PSUM") as PS,
            tc.tile_pool(name="small", bufs=4) as SM,
        ):
            # ---------- persistent loads ----------
            # adjT is 8MB: split across 4 engines' DMA queues so the first
            # head's work isn't gated on a single-queue 8MB transfer.
            adjT = P1.tile([128, JB, S], f32)
            adj_r = adjt_d[:].rearrange("(q jb p) i -> p q jb i", q=4, p=128)
            for q, eng in enumerate((nc.sync, nc.gpsimd, nc.scalar,
                                     nc.gpsimd)):
                eng.dma_start(out=adjT[:, q * 8:(q + 1) * 8, :],
                              in_=adj_r[:, q])
            # maskneg: 0 where adj>0, -1e30 where masked (additive pre-exp mask)
            mask = P1.tile([128, JB, S], b16)
            for q in range(4):
                nc.vector.tensor_scalar(
                    out=mask[:, q * 8:(q + 1) * 8, :],
                    in0=adjT[:, q * 8:(q + 1) * 8, :],
                    scalar1=0.0, scalar2=-1e30, op0=OP.is_le, op1=OP.mult)
            w1s = P1.tile([128, 2, H1, D1], b16)
            nc.sync.dma_start(out=w1s, in_=w1_d[:].rearrange(
                "(kb p) h d -> p kb h d", p=128))
            w2s = P1.tile([128, 4, H2, D2], b16)
            nc.sync.dma_start(out=w2s, in_=w2_d[:].rearrange(
                "(kb p) h d -> p kb h d", p=128))
            vel1s = P1.tile([128, 2, H1], b16)
            nc.sync.dma_start(out=vel1s, in_=vel1_d[:].rearrange(
                "(kb p) h -> p kb h", p=128))
            ver1s = P1.tile([128, 2, H1], b16)
            nc.sync.dma_start(out=ver1s, in_=ver1_d[:].rearrange(
                "(kb p) h -> p kb h", p=128))
            vel2s = P1.tile([128, 4, H2], b16)
            nc.sync.dma_start(out=vel2s, in_=vel2_d[:].rearrange(
                "(kb p) h -> p kb h", p=128))
            ver2s = P1.tile([128, 4, H2], b16)
            nc.sync.dma_start(out=ver2s, in_=ver2_d[:].rearrange(
                "(kb p) h -> p kb h", p=128))
            acbc = P1.tile([128, 2, H1 + H2], f32)
            nc.gpsimd.dma_start(out=acbc, in_=bcast_ap(acbc_d[:]))
            ident = P1.tile([128, 128], f32)
            make_identity(nc, ident)

            h1s = P1.tile([128, IB, F2], f32)      # layer-1 output slice

            # collective bounce buffers (partition-major pieces: node=lb*128+p)
            gins = [DR.tile([128, 4, 258], b16, name=f"gin{h}")
                    for h in range(H2)]
            gouts = [DR.tile([NC, 128, 4, 258], b16, addr_space="Shared",
                             name=f"gout{h}") for h in range(H2)]
            er2g = DR.tile([NC, 128, 4, H2], f32, addr_space="Shared")
            el2d = DR.tile([H2, S], b16)

            def attention(layer, h, haug, elbc, er_scalar_of, D, out_cb):
                """dense attention for one head; haug [128, JB, >=D+1] bf16 with
                ones at col D; er_scalar_of(jb) -> [128,1] AP; out_cb(ib, pacc_t)"""
                hi = h if layer == 1 else H1 + h
                pacc_t = [PACC.tile([128, D + 1], f32, name=f"pa{layer}_{h}_{ib}",
                                    tag="pacc") for ib in range(IB)]
                # half-chunks of 4 key blocks in SEPARATE tiles: each stage
                # (Prelu -> e-build -> mul -> exp -> 16 MMs) releases to the
                # next engine at half the latency, keeping the PE fed.
                for cg in range(NCH):
                    for hf in range(2):
                        j0 = cg * CH + hf * 4
                        mt = SM.tile([128, 4, S], b16, name=f"mt{hf}",
                                     tag=f"mt{hf}", bufs=2)
                        nc.scalar.activation(
                            out=mt, in_=adjT[:, j0:j0 + 4, :],
                            func=AF.Prelu,
                            bias=acbc[:, 1, hi:hi + 1],
                            scale=acbc[:, 0, hi:hi + 1], alpha=NEG)
                        et = SM.tile([128, 4, S], b16, name=f"et{hf}",
                                     tag=f"et{hf}", bufs=2)
                        for j4 in range(4):
                            jb = j0 + j4
                            nc.vector.scalar_tensor_tensor(
                                out=et[:, j4, :], in0=mask[:, jb, :],
                                scalar=er_scalar_of(jb), in1=elbc,
                                op0=OP.add, op1=OP.add)
                        nc.vector.tensor_mul(et, et, mt)
                        nc.scalar.activation(out=et, in_=et, func=AF.Exp)
                        # ib-outer: consecutive MMs per PSUM bank (HAM-friendly)
                        for ib in range(IB):
                            for j4 in range(4):
                                jb = j0 + j4
                                nc.tensor.matmul(
                                    pacc_t[ib][:, :],
                                    lhsT=et[:, j4, ib * 128:(ib + 1) * 128],
                                    rhs=haug[:, jb, 0:D + 1],
                                    start=(jb == 0), stop=(jb == JB - 1))
                for ib in range(IB):
                    out_cb(ib, pacc_t[ib])

            # =================== LAYER 1 ===================
            MID_cm = tc.tile_pool(name="mid", bufs=1)
            MID = MID_cm.__enter__()
            h1t = MID.tile([128, 4, S], b16, name="h1t", bufs=1)
            with (
                tc.tile_pool(name="l1", bufs=1) as L1,
                tc.tile_pool(name="haug1", bufs=2) as HA1,
            ):
                xts = L1.tile([128, 2, N], b16)
                xt_r = xt_d[:].rearrange("(kb p) n -> p kb n", p=128)
                nc.sync.dma_start(out=xts[:, 0, :], in_=xt_r[:, 0])
                nc.gpsimd.dma_start(out=xts[:, 1, :], in_=xt_r[:, 1])
                xtos = L1.tile([128, 2, S], b16)
                nc.sync.dma_start(out=xtos, in_=xto_d[:].rearrange(
                    "(kb p) n -> p kb n", p=128))

                # batched el/er for all 4 heads
                elall = L1.tile([H1, S], b16)
                pel = PS.tile([H1, S], f32, name="pel", tag="ps")
                for kb in range(2):
                    nc.tensor.matmul(pel, lhsT=vel1s[:, kb, :],
                                     rhs=xtos[:, kb, :],
                                     start=(kb == 0), stop=(kb == 1))
                nc.vector.tensor_copy(elall, pel)
                eld = DR.tile([H1, S], b16)
                nc.sync.dma_start(out=eld, in_=elall)
                # er in column layout [p, jb, h]: node jb*128+p, via PE
                ercol = L1.tile([128, JB, H1], f32)
                for g in range(8):
                    per = PS.tile([128, 4, H1], f32, name="per", tag="ps")
                    for j4 in range(4):
                        nb = g * 4 + j4
                        for kb in range(2):
                            nc.tensor.matmul(
                                per[:, j4, :],
                                lhsT=xts[:, kb, nb * 128:(nb + 1) * 128],
                                rhs=ver1s[:, kb, :],
                                start=(kb == 0), stop=(kb == 1))
                    nc.vector.tensor_copy(ercol[:, g * 4:(g + 1) * 4, :], per)

                for h in range(H1):
                    haug = HA1.tile([128, JB, D1 + 2], b16, name="haug",
                                    tag="haug")
                    nc.vector.memset(haug[:, :, D1:D1 + 1], 1.0)
                    # h_nat = x @ w1[h], written bf16 into haug cols 0:D1
                    for ng in range(8):
                        pn = PS.tile([128, 512], f32, name="pn", tag="ps")
                        for n4 in range(4):
                            nb = ng * 4 + n4
                            for kb in range(2):
                                nc.tensor.matmul(
                                    pn[:, n4 * 128:(n4 + 1) * 128],
                                    lhsT=xts[:, kb, nb * 128:(nb + 1) * 128],
                                    rhs=w1s[:, kb, h, :],
                                    start=(kb == 0), stop=(kb == 1))
                        src = pn[:].rearrange("p (a b) -> p a b", a=4)
                        dst = haug[:, ng * 4:(ng + 1) * 4, 0:D1]
                        nc.vector.tensor_copy(dst, src)
                    elbc = SM.tile([128, S], b16, name="elbc", tag="elbc",
                                   bufs=2)
                    nc.gpsimd.dma_start(out=elbc, in_=bcast_ap(eld[h]))

                    def l1_out(ib, pa, h=h):
                        rz = SM.tile([128, 1], f32, name="rz", tag="rz")
                        nc.vector.reciprocal(rz, pa[:, D1:D1 + 1])
                        tmp = SM.tile([128, D1], f32, name="tmp", tag="tmp")
                        nc.vector.tensor_scalar(
                            out=tmp, in0=pa[:, 0:D1], scalar1=rz, scalar2=None,
                            op0=OP.mult)
                        ex = SM.tile([128, D1], f32, name="ex", tag="ex")
                        nc.scalar.activation(out=ex, in_=tmp, func=AF.Exp)
                        nc.vector.tensor_scalar(
                            out=ex, in0=ex, scalar1=-1.0, scalar2=0.0,
                            op0=OP.add, op1=OP.min)
                        nc.vector.tensor_scalar(
                            out=tmp, in0=tmp, scalar1=0.0, scalar2=None,
                            op0=OP.max)
                        nc.vector.tensor_add(
                            h1s[:, ib, h * D1:(h + 1) * D1], ex, tmp)

                    attention(1, h, haug, elbc,
                              lambda jb, h=h: ercol[:, jb, h:h + 1],
                              D1, l1_out)
                    # transpose this head's [S, 128] output slice into h1t
                    for nb in range(4):
                        ptt = PS.tile([128, 128], f32, name="ptt", tag="ps")
                        nc.tensor.transpose(
                            ptt, h1s[:, nb, h * D1:(h + 1) * D1], ident)
                        nc.vector.tensor_copy(
                            h1t[:, h, nb * 128:(nb + 1) * 128], ptt)

            # ============ LAYER BOUNDARY: pieces + AllGather ============
            with tc.tile_pool(name="bnd", bufs=2) as BND:
                # batched el2/er2 for all 6 heads
                el2all = BND.tile([H2, S], b16, name="el2all", bufs=1)
                pe2 = PS.tile([H2, S], f32, name="pe2", tag="ps")
                for kb in range(4):
                    nc.tensor.matmul(pe2, lhsT=vel2s[:, kb, :],
                                     rhs=h1t[:, kb, :],
                                     start=(kb == 0), stop=(kb == 3))
                nc.vector.tensor_copy(el2all, pe2)
                nc.sync.dma_start(out=el2d, in_=el2all)
                # er2 piece in column layout [p, lb, h] (node lb*128+p)
                pr2 = PS.tile([128, 4, H2], f32, name="pr2", tag="ps")
                for nb in range(4):
                    for kb in range(4):
                        nc.tensor.matmul(
                            pr2[:, nb, :],
                            lhsT=h1t[:, kb, nb * 128:(nb + 1) * 128],
                            rhs=ver2s[:, kb, :],
                            start=(kb == 0), stop=(kb == 3))
                er2tmp = BND.tile([128, 4, H2], f32, name="er2tmp", bufs=1)
                nc.vector.tensor_copy(er2tmp, pr2)
                er2d = DR.tile([128, 4, H2], f32)
                nc.sync.dma_start(out=er2d, in_=er2tmp)
                nc.gpsimd.collective_compute(
                    "AllGather", mybir.AluOpType.bypass,
                    replica_groups=[list(range(NC))],
                    ins=[er2d.opt()], outs=[er2g.opt()])
                for h in range(H2):
                    pc = BND.tile([128, 4, 258], b16, name="pc", tag="pc")
                    nc.vector.memset(pc[:, :, 256:257], 1.0)
                    for nb in range(4):
                        pp = PS.tile([128, D2], f32, name="pp", tag="ps")
                        for kb in range(4):
                            nc.tensor.matmul(
                                pp, lhsT=h1t[:, kb, nb * 128:(nb + 1) * 128],
                                rhs=w2s[:, kb, h, :],
                                start=(kb == 0), stop=(kb == 3))
                        nc.vector.tensor_copy(pc[:, nb, 0:D2], pp)
                    nc.sync.dma_start(out=gins[h], in_=pc)
                    nc.gpsimd.collective_compute(
                        "AllGather", mybir.AluOpType.bypass,
                        replica_groups=[list(range(NC))],
                        ins=[gins[h].opt()], outs=[gouts[h].opt()])
            MID_cm.__exit__(None, None, None)

            # =================== LAYER 2 ===================
            with tc.tile_pool(name="haug2", bufs=2) as HA2:
                acc = HA2.tile([128, IB, D2], f32, name="acc", bufs=1)
                er2all = HA2.tile([128, JB, H2], f32, name="er2all", bufs=1)
                nc.gpsimd.dma_start(
                    out=er2all[:].rearrange("p (c lb) h -> p c lb h", lb=4),
                    in_=er2g[:].rearrange("c p lb h -> p c lb h"))
                for h in range(H2):
                    aug2 = HA2.tile([128, JB, 258], b16, name="aug2",
                                    tag="aug2")
                    nc.sync.dma_start(
                        out=aug2[:].rearrange("p (c lb) col -> p c lb col",
                                              lb=4),
                        in_=gouts[h][:].rearrange(
                            "c p lb col -> p c lb col"))
                    elbc2 = SM.tile([128, S], b16, name="elbc2", tag="elbc",
                                    bufs=2)
                    nc.gpsimd.dma_start(out=elbc2, in_=bcast_ap(el2d[h]))

                    def l2_out(ib, pa, h=h):
                        rz = SM.tile([128, 1], f32, name="rz2", tag="rz")
                        nc.vector.reciprocal(rz, pa[:, D2:D2 + 1])
                        if h == 0:
                            nc.vector.tensor_scalar(
                                out=acc[:, ib, :], in0=pa[:, 0:D2],
                                scalar1=rz, scalar2=None, op0=OP.mult)
                        else:
                            nc.vector.scalar_tensor_tensor(
                                out=acc[:, ib, :], in0=pa[:, 0:D2],
                                scalar=rz, in1=acc[:, ib, :],
                                op0=OP.mult, op1=OP.add)

                    attention(2, h, aug2, elbc2,
                              lambda jb, h=h: er2all[:, jb, h:h + 1],
                              D2, l2_out)

                # ============ epilogue: mean, elu, node-max ============
                oloc = HA2.tile([128, IB, D2], f32, name="oloc", bufs=1)
                omax_p = HA2.tile([128, 2, IB], f32, name="omax_p", bufs=1)
                omax = HA2.tile([128, 2], f32, name="omax", bufs=1)
                for ib in range(IB):
                    ex = SM.tile([128, D2], f32, name="ex2", tag="tmp")
                    nc.scalar.activation(out=ex, in_=acc[:, ib, :],
                                         func=AF.Exp, scale=1.0 / H2)
                    nc.vector.tensor_scalar(out=ex, in0=ex, scalar1=-1.0,
                                            scalar2=0.0, op0=OP.add,
                                            op1=OP.min)
                    t2 = SM.tile([128, D2], f32, name="t2", tag="ex")
                    nc.vector.tensor_scalar(out=t2, in0=acc[:, ib, :],
                                            scalar1=1.0 / H2, scalar2=0.0,
                                            op0=OP.mult, op1=OP.max)
                    nc.vector.tensor_add(oloc[:, ib, :], ex, t2)
                nc.sync.dma_start(
                    out=oloc_d[:].rearrange("(ib p) d -> p ib d", p=128),
                    in_=oloc)
                for ib in range(IB):
                    for dh in range(2):
                        ptt = PS.tile([128, 128], f32, name="ptt2", tag="ps")
                        nc.tensor.transpose(
                            ptt, oloc[:, ib, dh * 128:(dh + 1) * 128], ident)
                        nc.vector.tensor_reduce(
                            out=omax_p[:, dh, ib:ib + 1], in_=ptt,
                            axis=AX.X, op=OP.max)
                for dh in range(2):
                    nc.vector.tensor_reduce(
                        out=omax[:, dh:dh + 1], in_=omax_p[:, dh, :],
                        axis=AX.X, op=OP.max)
                nc.sync.dma_start(out=omax_d[:].rearrange("a p -> p a"),
                                  in_=omax)

    nc.compile()
    return nc


def _get_built():
    global _BUILT
    if _BUILT is None:
        _BUILT = _build()
    return _BUILT


def _marshal(x, adj, w1, a1, w2, a2):
    x0 = np.asarray(x, np.float32)[0]
    adj = np.asarray(adj, np.float32)
    w1 = np.asarray(w1, np.float32)
    a1 = np.asarray(a1, np.float32)
    w2 = np.asarray(w2, np.float32)
    a2 = np.asarray(a2, np.float32)
    xt = np.ascontiguousarray(x0.T).astype(bf)
    w1t = np.ascontiguousarray(np.transpose(w1, (1, 0, 2))).astype(bf)
    w2t = np.ascontiguousarray(np.transpose(w2, (1, 0, 2))).astype(bf)
    vel1 = np.einsum('hfd,hd->fh', w1, a1[:, :D1]).astype(bf)
    ver1 = np.einsum('hfd,hd->fh', w1, a1[:, D1:]).astype(bf)
    vel2 = np.einsum('hfd,hd->fh', w2, a2[:, :D2]).astype(bf)
    ver2 = np.einsum('hfd,hd->fh', w2, a2[:, D2:]).astype(bf)
    return x0, adj, xt, w1t, w2t, vel1, ver1, vel2, ver2


def run(trace=False, **inputs):
    from concourse.bass_utils import run_bass_kernel_spmd
    nc = _get_built()
    x0, adj, xt, w1t, w2t, vel1, ver1, vel2, ver2 = _marshal(
        inputs['x'], inputs['adj'], inputs['w1'], inputs['a1'],
        inputs['w2'], inputs['a2'])
    acbc = np.stack([
        np.concatenate([np.asarray(inputs['ac1'], np.float32),
                        np.asarray(inputs['ac2'], np.float32)]),
        np.concatenate([np.asarray(inputs['bc1'], np.float32),
                        np.asarray(inputs['bc2'], np.float32)]),
    ]).astype(np.float32)
    in_maps = []
    for c in range(NC):
        in_maps.append({
            'adjt': np.ascontiguousarray(adj[c * S:(c + 1) * S, :].T),
            'xt': xt,
            'xto': np.ascontiguousarray(xt[:, c * S:(c + 1) * S]),
            'w1t': w1t, 'w2t': w2t,
            'vel1': vel1, 'ver1': ver1, 'vel2': vel2, 'ver2': ver2,
            'acbc': acbc,
        })
    kw = {}
    if trace:
        kw = dict(trace=True, trace_cores=[0])
    res = run_bass_kernel_spmd(nc, in_maps, core_ids=list(range(NC)), **kw)
    omax = np.max(np.stack([r['omax'] for r in res.results]), axis=0)
    omax = omax.reshape(D2)
    out = (omax @ np.asarray(inputs['Wm'], np.float32)
           + np.asarray(inputs['bm'], np.float32))[None, :]
    return out.astype(np.float32), res


def kernel(**inputs) -> np.ndarray:
    out, _ = run(trace=False, **inputs)
    return out
